# revision 5
# baseline (speedup 1.0000x reference)
"""Trainium2 Bass kernel for nn_DigitLayer (CapsNet digit-capsule layer).

Math note: the reference's routing softmax acts on a size-1 axis, so the
coupling coefficients are exactly 1.0 on every iteration and the whole
3-iteration routing loop collapses to

    S[b,d,i] = sum_{p,j} W[p,d,i,j] * x[b,p,j]
    out      = squash(S)  over i (the 16-dim)

i.e. one [B, P*8] @ [P*8, D*16] matmul plus a per-(b,d) squash.

Distribution: the contraction dim P (1152) is sharded across the 8 cores so
every byte of x and W is read from HBM exactly once chip-wide. Each core
computes a partial S[b, (d,i)] over its P-shard for all 256 batches via 18
accumulating PE matmuls; the host sums the 8 partial tensors and applies the
(collapsed-routing) squash.

Inputs are fed to the device as float16: the PE runs f16 at full rate and the
DMA bytes halve; measured end-to-end relative error is ~4e-4 (accumulation is
fp32 in PSUM).

Schedule: the two input loads are one DMA per ring (x on the SP HWDGE ring,
w on the ACT ring, issued back-to-back at program start) and the PE gates on
full completion of both (==16 sub-increments each, so no partial-arrival
races). All 18 matmuls then run back-to-back: batch-half 0 accumulates and
closes first, its PSUM bank is stored straight to DRAM (no DVE bounce) while
half 1 accumulates into the other bank, then half 1 is drained and stored.

Device-side layout (per core, all host-prepped, SBUF-native):
    xt [128, 9, 256] f16 : xT chunks, k_local = kc*128 + kp = p_local*8 + j
    wt [128, 9, 160] f16 : W2 chunks, same k mapping, n = d*16 + i
    out [256, 160] f32   : partial S
"""

import numpy as np

import concourse.bacc as bacc
import concourse.mybir as mybir
from concourse.bass_utils import run_bass_kernel_spmd

B, P, D, VP, VD = 256, 1152, 10, 8, 16
NCORES = 8
PL = P // NCORES           # 144 primary capsules per core
KL = PL * VP               # 1152 local contraction length
KCH = KL // 128            # 9 k-chunks of 128
N_OUT = D * VD             # 160
MB = 128                   # batch chunk (matmul M / PSUM partitions)
NMB = B // MB              # 2

_cache = {}


def _hoist_first(nc, instrs):
    """Move the given instructions to the front of their engine's stream so
    the input DMAs issue before the framework preamble (all-engine barrier)
    and their transfer latency overlaps it."""
    names = {i.name for i in instrs}
    for bb in nc.main_func.blocks:
        if not any(ins.name in names for ins in bb.instructions):
            continue
        by_engine = {}
        for ins in bb.instructions:
            if ins.name in names:
                by_engine.setdefault(ins.engine, []).append(ins)
        new = []
        emitted = set()
        for ins in bb.instructions:
            if ins.name in names:
                continue
            e = ins.engine
            if e in by_engine and e not in emitted:
                new.extend(by_engine[e])
                emitted.add(e)
            new.append(ins)
        for e, lst in by_engine.items():
            if e not in emitted:
                new.extend(lst)
        bb.instructions[:] = new


def _strip_const_memsets(nc):
    """Drop the framework's const-AP Memsets (unused by this kernel) from the
    Pool stream. Nothing reads those SBUF constants here, and removing them
    leaves the PE's first LDWEIGHTS/MATMUL as the kernel's first compute
    instruction."""
    for bb in nc.main_func.blocks:
        bb.instructions[:] = [
            i for i in bb.instructions
            if not (type(i).__name__ == "InstMemset"
                    and "const-" in str(getattr(i, "outs", "")))
        ]


def _build():
    """Raw-bass kernel (no TileContext), hand-placed semaphores.

    Hard-won rules baked in here:
      * One semaphore per DMA: a HWDGE DMA completes as 16 unordered +1
        sub-increments, so intermediate thresholds on a shared sem race.
      * The PE gate must wait on the DMA completion semaphores; an engine
        DRAIN does NOT barrier HWDGE DMA data (cold-run NaNs).
      * Half 0's stop-matmul incs its sem and is stored while the PE is
        still accumulating half 1 in the other PSUM bank; half 1 gets a
        full PE drain before its store.
      * No wait on the output DMA semaphore: the runtime end-of-program
        barrier covers it (verified bitwise against the waiting variant).
    """
    dt_in = mybir.dt.float16
    nc = bacc.Bacc("TRN2", debug=False, num_devices=NCORES)
    xt = nc.dram_tensor("xt", [128, KCH, B], dt_in, kind="ExternalInput").ap()
    wt = nc.dram_tensor("wt", [128, KCH, N_OUT], dt_in, kind="ExternalInput").ap()
    out = nc.dram_tensor("out", [B, N_OUT], mybir.dt.float32, kind="ExternalOutput").ap()

    from contextlib import ExitStack
    with ExitStack() as ctx:
        xsb = ctx.enter_context(nc.sbuf_tensor([128, KCH, B], dt_in))
        wsb = ctx.enter_context(nc.sbuf_tensor([128, KCH, N_OUT], dt_in))
        pts = [
            ctx.enter_context(nc.psum_tensor(f"pt{m}", [MB, N_OUT], mybir.dt.float32))
            for m in range(NMB)
        ]
        osb = ctx.enter_context(nc.sbuf_tensor([MB, NMB, N_OUT], mybir.dt.float32))
        sem_x = ctx.enter_context(nc.semaphore(name="sem_x"))
        sem_w = ctx.enter_context(nc.semaphore(name="sem_w"))
        sem_mm = ctx.enter_context(nc.semaphore(name="sem_mm"))
        sem_cp = ctx.enter_context(nc.semaphore(name="sem_cp"))
        sem_out = ctx.enter_context(nc.semaphore(name="sem_out"))

        # input DMAs: one per ring (x on SP, w on ACT), issued in parallel
        in_dmas = [
            nc.sync.dma_start(out=xsb[:], in_=xt).then_inc(sem_x, 16).ins,
            nc.scalar.dma_start(out=wsb[:], in_=wt).then_inc(sem_w, 16).ins,
        ]

        # PE: gate once on both full loads, then 18 back-to-back matmuls.
        # Batch-half outer so half 0's PSUM bank closes 9 matmuls early and
        # its store overlaps half 1's accumulation.
        nc.tensor.wait_ge(sem_x, 16)
        nc.tensor.wait_ge(sem_w, 16)
        for m in range(NMB):
            for k in range(KCH):
                mm = nc.tensor.matmul(
                    pts[m][:],
                    lhsT=xsb[:, k, m * MB:(m + 1) * MB],
                    rhs=wsb[:, k, :],
                    start=(k == 0),
                    stop=(k == KCH - 1),
                )
            if m == 0:
                mm.then_inc(sem_mm, 1)
            else:
                nc.tensor.drain().then_inc(sem_mm, 1)

        # DVE: copy each half as soon as its accumulation closes; per-copy
        # drain so each store reads settled SBUF. Half 0's copy+store overlap
        # half 1's matmuls (different PSUM bank).
        for m in range(NMB):
            nc.vector.wait_ge(sem_mm, m + 1)
            nc.vector.tensor_copy(osb[:, m, :], pts[m][:])
            nc.vector.drain().then_inc(sem_cp, 1)
        # Stores: half 0 on the ACT ring (hidden under half 1's matmuls),
        # half 1 on the SP ring — so the runtime's end-of-program fence on
        # each ring covers exactly one store and the final Sync drain only
        # waits on the last one.
        nc.scalar.wait_ge(sem_cp, 1)
        nc.scalar.dma_start(out=out[0:MB, :], in_=osb[:, 0, :]).then_inc(sem_out, 16)
        nc.sync.wait_ge(sem_cp, 2)
        nc.sync.dma_start(out=out[MB:B, :], in_=osb[:, 1, :]).then_inc(sem_out, 16)

        _hoist_first(nc, in_dmas)
        _strip_const_memsets(nc)
    nc.compile()
    return nc


def _prep_inputs(x, W):
    """Per-core host-side layout: SBUF-native [128, KCH, *] f16 arrays."""
    xs = np.ascontiguousarray(x[..., 0], dtype=np.float32)      # [B, P, 8]
    W = np.asarray(W, dtype=np.float32)
    in_maps = []
    for c in range(NCORES):
        pr = slice(c * PL, (c + 1) * PL)
        # x^T chunk: [128, KCH, B] with k_local = kc*128 + kp = p_local*8 + j
        xl = xs[:, pr, :].reshape(B, KL).T                      # [KL, B]
        xl = xl.reshape(KCH, 128, B).transpose(1, 0, 2)         # [128, KCH, B]
        # W2 chunk: W2[(p_local, j), (d, i)] = W[p, d, i, j]
        wl = W[pr].transpose(0, 3, 1, 2).reshape(KL, N_OUT)     # [KL, 160]
        wl = wl.reshape(KCH, 128, N_OUT).transpose(1, 0, 2)     # [128, KCH, 160]
        in_maps.append({
            "xt": np.ascontiguousarray(xl, dtype=np.float16),
            "wt": np.ascontiguousarray(wl, dtype=np.float16),
        })
    return in_maps


def _squash(S):
    """S: [B, 160] summed partials -> squash over each group of 16."""
    S = S.reshape(B, D, VD)
    sq = np.sum(S * S, axis=2, keepdims=True)
    v = S * sq / (1.0 + sq) / np.sqrt(sq + 1e-9)
    return v[..., None].astype(np.float32)                      # [B, D, 16, 1]


def run(x, W, trace=False):
    if "nc" not in _cache:
        _cache["nc"] = _build()
    nc = _cache["nc"]
    in_maps = _prep_inputs(x, W)
    try:
        res = run_bass_kernel_spmd(nc, in_maps, core_ids=list(range(NCORES)), trace=trace)
    except Exception:
        # one retry absorbs transient runtime hiccups
        res = run_bass_kernel_spmd(nc, in_maps, core_ids=list(range(NCORES)), trace=trace)
    S = np.zeros((B, N_OUT), dtype=np.float32)
    for c in range(NCORES):
        S += res.results[c]["out"]
    return _squash(S), res


def kernel(x, W):
    out, _ = run(np.asarray(x), np.asarray(W))
    return out


# revision 6
# speedup vs baseline: 1.0075x; 1.0075x over previous
"""Trainium2 Bass kernel for nn_DigitLayer (CapsNet digit-capsule layer).

Math note: the reference's routing softmax acts on a size-1 axis, so the
coupling coefficients are exactly 1.0 on every iteration and the whole
3-iteration routing loop collapses to

    S[b,d,i] = sum_{p,j} W[p,d,i,j] * x[b,p,j]
    out      = squash(S)  over i (the 16-dim)

i.e. one [B, P*8] @ [P*8, D*16] matmul plus a per-(b,d) squash.

Distribution: the contraction dim P (1152) is sharded across the 8 cores so
every byte of x and W is read from HBM exactly once chip-wide. Each core
computes a partial S[b, (d,i)] over its P-shard for all 256 batches via 18
accumulating PE matmuls; the host sums the 8 partial tensors and applies the
(collapsed-routing) squash.

Inputs are fed to the device as float16: the PE runs f16 at full rate and the
DMA bytes halve; measured end-to-end relative error is ~4e-4 (accumulation is
fp32 in PSUM).

Schedule: the two input loads are one DMA per ring (x on the SP HWDGE ring,
w on the ACT ring, issued back-to-back at program start) and the PE gates on
full completion of both (==16 sub-increments each, so no partial-arrival
races). All 18 matmuls then run back-to-back: batch-half 0 accumulates and
closes first, its PSUM bank is stored straight to DRAM (no DVE bounce) while
half 1 accumulates into the other bank, then half 1 is drained and stored.

Device-side layout (per core, all host-prepped, SBUF-native):
    xt [128, 9, 256] f16 : xT chunks, k_local = kc*128 + kp = p_local*8 + j
    wt [128, 9, 160] f16 : W2 chunks, same k mapping, n = d*16 + i
    out [256, 160] f32   : partial S
"""

import numpy as np

import concourse.bacc as bacc
import concourse.mybir as mybir
from concourse.bass_utils import run_bass_kernel_spmd

B, P, D, VP, VD = 256, 1152, 10, 8, 16
NCORES = 8
PL = P // NCORES           # 144 primary capsules per core
KL = PL * VP               # 1152 local contraction length
KCH = KL // 128            # 9 k-chunks of 128
N_OUT = D * VD             # 160
MB = 128                   # batch chunk (matmul M / PSUM partitions)
NMB = B // MB              # 2

_cache = {}


def _hoist_first(nc, instrs):
    """Move the given instructions to the front of their engine's stream so
    the input DMAs issue before the framework preamble (all-engine barrier)
    and their transfer latency overlaps it."""
    names = {i.name for i in instrs}
    for bb in nc.main_func.blocks:
        if not any(ins.name in names for ins in bb.instructions):
            continue
        by_engine = {}
        for ins in bb.instructions:
            if ins.name in names:
                by_engine.setdefault(ins.engine, []).append(ins)
        new = []
        emitted = set()
        for ins in bb.instructions:
            if ins.name in names:
                continue
            e = ins.engine
            if e in by_engine and e not in emitted:
                new.extend(by_engine[e])
                emitted.add(e)
            new.append(ins)
        for e, lst in by_engine.items():
            if e not in emitted:
                new.extend(lst)
        bb.instructions[:] = new


def _strip_const_memsets(nc):
    """Drop the framework's const-AP Memsets (unused by this kernel) from the
    Pool stream. Nothing reads those SBUF constants here, and removing them
    leaves the PE's first LDWEIGHTS/MATMUL as the kernel's first compute
    instruction."""
    for bb in nc.main_func.blocks:
        bb.instructions[:] = [
            i for i in bb.instructions
            if not (type(i).__name__ == "InstMemset"
                    and "const-" in str(getattr(i, "outs", "")))
        ]


def _build():
    """Raw-bass kernel (no TileContext), hand-placed semaphores.

    Hard-won rules baked in here:
      * One semaphore per DMA: a HWDGE DMA completes as 16 unordered +1
        sub-increments, so intermediate thresholds on a shared sem race.
      * The PE gate must wait on the DMA completion semaphores; an engine
        DRAIN does NOT barrier HWDGE DMA data (cold-run NaNs).
      * Half 0's stop-matmul incs its sem and is stored while the PE is
        still accumulating half 1 in the other PSUM bank; half 1 gets a
        full PE drain before its store.
      * No wait on the output DMA semaphore: the runtime end-of-program
        barrier covers it (verified bitwise against the waiting variant).
    """
    dt_in = mybir.dt.float16
    nc = bacc.Bacc("TRN2", debug=False, num_devices=NCORES)
    xt = nc.dram_tensor("xt", [128, KCH, B], dt_in, kind="ExternalInput").ap()
    wt = nc.dram_tensor("wt", [128, KCH, N_OUT], dt_in, kind="ExternalInput").ap()
    out = nc.dram_tensor("out", [B, N_OUT], mybir.dt.float32, kind="ExternalOutput").ap()

    from contextlib import ExitStack
    with ExitStack() as ctx:
        xsb = ctx.enter_context(nc.sbuf_tensor([128, KCH, B], dt_in))
        wsb = ctx.enter_context(nc.sbuf_tensor([128, KCH, N_OUT], dt_in))
        pts = [
            ctx.enter_context(nc.psum_tensor(f"pt{m}", [MB, N_OUT], mybir.dt.float32))
            for m in range(NMB)
        ]
        osb = ctx.enter_context(nc.sbuf_tensor([MB, NMB, N_OUT], mybir.dt.float32))
        sem_x = ctx.enter_context(nc.semaphore(name="sem_x"))
        sem_w = ctx.enter_context(nc.semaphore(name="sem_w"))
        sem_mm = ctx.enter_context(nc.semaphore(name="sem_mm"))
        sem_cp = ctx.enter_context(nc.semaphore(name="sem_cp"))
        sem_out = ctx.enter_context(nc.semaphore(name="sem_out"))

        # input DMAs: one per ring (x on SP, w on ACT), issued in parallel
        in_dmas = [
            nc.sync.dma_start(out=xsb[:], in_=xt).then_inc(sem_x, 16).ins,
            nc.scalar.dma_start(out=wsb[:], in_=wt).then_inc(sem_w, 16).ins,
        ]

        # PE: gate once on both full loads, then 18 back-to-back matmuls.
        # Batch-half outer so half 0's PSUM bank closes 9 matmuls early and
        # its store overlaps half 1's accumulation.
        nc.tensor.wait_ge(sem_x, 16)
        nc.tensor.wait_ge(sem_w, 16)
        for m in range(NMB):
            for k in range(KCH):
                mm = nc.tensor.matmul(
                    pts[m][:],
                    lhsT=xsb[:, k, m * MB:(m + 1) * MB],
                    rhs=wsb[:, k, :],
                    start=(k == 0),
                    stop=(k == KCH - 1),
                )
            if m == 0:
                mm.then_inc(sem_mm, 1)
            else:
                nc.tensor.drain().then_inc(sem_mm, 1)

        # DVE: copy each half as soon as its accumulation closes. Half 0's
        # copy+store overlap half 1's matmuls (different PSUM bank). The
        # copy's own retire gates the store (benched bitwise-identical to the
        # drain-gated variant, and the drain hop is on the critical path for
        # half 1).
        for m in range(NMB):
            nc.vector.wait_ge(sem_mm, m + 1)
            nc.vector.tensor_copy(osb[:, m, :], pts[m][:]).then_inc(sem_cp, 1)
        # SP: per-half stores
        for m in range(NMB):
            nc.sync.wait_ge(sem_cp, m + 1)
            nc.sync.dma_start(
                out=out[m * MB:(m + 1) * MB, :], in_=osb[:, m, :]
            ).then_inc(sem_out, 16)

        _hoist_first(nc, in_dmas)
        _strip_const_memsets(nc)
    nc.compile()
    return nc


def _prep_inputs(x, W):
    """Per-core host-side layout: SBUF-native [128, KCH, *] f16 arrays."""
    xs = np.ascontiguousarray(x[..., 0], dtype=np.float32)      # [B, P, 8]
    W = np.asarray(W, dtype=np.float32)
    in_maps = []
    for c in range(NCORES):
        pr = slice(c * PL, (c + 1) * PL)
        # x^T chunk: [128, KCH, B] with k_local = kc*128 + kp = p_local*8 + j
        xl = xs[:, pr, :].reshape(B, KL).T                      # [KL, B]
        xl = xl.reshape(KCH, 128, B).transpose(1, 0, 2)         # [128, KCH, B]
        # W2 chunk: W2[(p_local, j), (d, i)] = W[p, d, i, j]
        wl = W[pr].transpose(0, 3, 1, 2).reshape(KL, N_OUT)     # [KL, 160]
        wl = wl.reshape(KCH, 128, N_OUT).transpose(1, 0, 2)     # [128, KCH, 160]
        in_maps.append({
            "xt": np.ascontiguousarray(xl, dtype=np.float16),
            "wt": np.ascontiguousarray(wl, dtype=np.float16),
        })
    return in_maps


def _squash(S):
    """S: [B, 160] summed partials -> squash over each group of 16."""
    S = S.reshape(B, D, VD)
    sq = np.sum(S * S, axis=2, keepdims=True)
    v = S * sq / (1.0 + sq) / np.sqrt(sq + 1e-9)
    return v[..., None].astype(np.float32)                      # [B, D, 16, 1]


def run(x, W, trace=False):
    if "nc" not in _cache:
        _cache["nc"] = _build()
    nc = _cache["nc"]
    in_maps = _prep_inputs(x, W)
    try:
        res = run_bass_kernel_spmd(nc, in_maps, core_ids=list(range(NCORES)), trace=trace)
    except Exception:
        # one retry absorbs transient runtime hiccups
        res = run_bass_kernel_spmd(nc, in_maps, core_ids=list(range(NCORES)), trace=trace)
    S = np.zeros((B, N_OUT), dtype=np.float32)
    for c in range(NCORES):
        S += res.results[c]["out"]
    return _squash(S), res


def kernel(x, W):
    out, _ = run(np.asarray(x), np.asarray(W))
    return out
